# revision 5
# baseline (speedup 1.0000x reference)
"""FourierFT fused kernel for Trainium2 (8 NeuronCores, SPMD data-parallel).

Computes h = x @ W_base^T + b_base + x @ Delta_W where
Delta_W = real(ifft2(scatter(c, E))) * ALPHA.

Key algebraic identity: with only N=100 nonzero spectral coefficients,
Delta_W[k, l] = s * sum_j c_j * cos(2*pi*(k*u_j + l*v_j)/4096)
             = (A @ B)[k, l]   with rank 2N:
  A[k, j]    =  c_j*s*cos(2*pi*k*u_j/4096)     B[j, l]    = cos(2*pi*l*v_j/4096)
  A[k, N+j]  = -c_j*s*sin(2*pi*k*u_j/4096)     B[N+j, l]  = sin(2*pi*l*v_j/4096)
so the dense iFFT is never materialized; the delta path is a rank-200 update
folded into the same PSUM accumulation as the base matmul.

Device layout: each core owns a 1024-row slice of x (flattened to [8192, 4096]),
pre-transposed on the host to k-major ([4096, 1024]) so the contraction dim sits
on SBUF partitions. W_base is host-transposed to [in, out] and replicated. The
output is produced as h^T tiles ([l, s]) so the bias is a per-partition scalar
and DMA rows stay 4 KB-contiguous; the host re-transposes the shards at the end.

Trig tables A and B are built on-device: the index products k*u and l*v stay
below 2^24 so all f32 index arithmetic is exact; range reduction to the Sin
LUT's [-pi, pi] window uses the round-to-nearest magic constant 2^23.
Matmuls run as float32r (FP22 multiplies, fp32 accumulate) which streams at
full PE rate for moving dims >= 256; B and t = x@A are kept in bf16 (their
contribution to h is ~0.6%, so bf16 there perturbs h by ~1e-5 relative).
"""

import sys

if "/opt/trn_rl_repo" not in sys.path:
    sys.path.insert(0, "/opt/trn_rl_repo")

import numpy as np

import concourse.bass as bass  # noqa: F401  (registers AP machinery)
import concourse.mybir as mybir
import concourse.tile as tile
from concourse import bacc, bass_utils

D1 = 4096
D2 = 4096
ALPHA = 300.0
NCOEF = 100
NCORES = 8
S_TOTAL = 4 * 2048
S_CORE = S_TOTAL // NCORES  # 1024
KT = D1 // 128  # 32 k-tiles
R = 256  # padded rank (cols 0..99 cos, 100..199 sin, rest zero)
MAGIC = float(2**23)
# one ulp below 2*pi: keeps f*scale inside the Sin LUT's [-pi, pi] window
# even at f = +/-0.5 exactly (f32(pi) rounds above float64 pi)
TWO_PI = float(np.nextafter(np.float32(2 * np.pi), np.float32(0)))
INV4096 = float(2**-12)

F32 = mybir.dt.float32
F32R = mybir.dt.float32r
BF16 = mybir.dt.bfloat16
MULT = mybir.AluOpType.mult
ADD = mybir.AluOpType.add
SUB = mybir.AluOpType.subtract
SIN = mybir.ActivationFunctionType.Sin
IDENT = mybir.ActivationFunctionType.Identity

_CACHE = {}


def _build_nc():
    """Trace + compile the single-core program (identical across cores)."""
    nc = bacc.Bacc("TRN2", target_bir_lowering=False, debug=False)

    xt_d = nc.dram_tensor("xt", [D1, S_CORE], F32R, kind="ExternalInput").ap()
    wt_d = nc.dram_tensor("wt", [D1, D2], F32R, kind="ExternalInput").ap()
    bias_d = nc.dram_tensor("biasc", [128, 32], F32, kind="ExternalInput").ap()
    u2_d = nc.dram_tensor("u2", [128, R], F32, kind="ExternalInput").ap()
    puq_d = nc.dram_tensor("puq", [128, R], F32, kind="ExternalInput").ap()
    mcols_d = nc.dram_tensor("mcols", [128, 2], F32, kind="ExternalInput").ap()
    vcols_d = nc.dram_tensor("vcols", [128, 2], F32, kind="ExternalInput").ap()
    offc_d = nc.dram_tensor("offc", [128, 16], F32, kind="ExternalInput").ap()
    lrow_d = nc.dram_tensor("lrow", [128, 512], F32, kind="ExternalInput").ap()
    ht_d = nc.dram_tensor("ht", [D2, S_CORE], F32, kind="ExternalOutput").ap()

    with tile.TileContext(nc) as tc:
        with (
            tc.tile_pool(name="resident", bufs=1) as rpool,
            tc.tile_pool(name="wstream", bufs=6) as wpool,
            tc.tile_pool(name="ascratch", bufs=2) as apool,
            tc.tile_pool(name="bscratch", bufs=2) as bpool,
            tc.tile_pool(name="outstage", bufs=3) as opool,
            tc.tile_pool(name="psum", bufs=8, space="PSUM") as ppool,
        ):
            # ---- resident tiles ----
            xt_sb = rpool.tile([128, KT, S_CORE], F32R, tag="xt")
            b_sb = rpool.tile([128, 2, D2], BF16, tag="bmat")
            tt_sb = rpool.tile([128, 2, S_CORE], BF16, tag="tt")
            lrow_sb = rpool.tile([128, 512], F32, tag="lrow")
            u2_sb = rpool.tile([128, R], F32, tag="u2")
            puq_sb = rpool.tile([128, R], F32, tag="puq")
            bias_sb = rpool.tile([128, 32], F32, tag="bias")
            mcols_sb = rpool.tile([128, 2], F32, tag="mcols")
            vcols_sb = rpool.tile([128, 2], F32, tag="vcols")
            offc_sb = rpool.tile([128, 16], F32, tag="offc")

            nc.sync.dma_start(u2_sb[:], u2_d[:])
            nc.sync.dma_start(puq_sb[:], puq_d[:])
            nc.sync.dma_start(mcols_sb[:], mcols_d[:])
            nc.sync.dma_start(vcols_sb[:], vcols_d[:])
            nc.sync.dma_start(offc_sb[:], offc_d[:])
            nc.sync.dma_start(lrow_sb[:], lrow_d[:])
            nc.sync.dma_start(bias_sb[:], bias_d[:])
            for kt in range(KT):
                nc.sync.dma_start(
                    xt_sb[:, kt, :], xt_d[kt * 128 : (kt + 1) * 128, :]
                )

            # ---- prefix: A tiles (on the fly) -> t^T = A^T-chunks @ x^T ----
            pmt = [[ppool.tile([128, 512], F32, tag="pm", name=f"pmt_{r}_{h}")
                    for h in range(2)] for r in range(2)]
            for kt in range(KT):
                x2 = apool.tile([128, R], F32, tag="ax")
                nc.vector.scalar_tensor_tensor(
                    x2, u2_sb[:], float(kt * 128) * INV4096, puq_sb[:], MULT, ADD
                )
                z2 = apool.tile([128, R], F32, tag="az")
                nc.vector.tensor_scalar(z2, x2, MAGIC, MAGIC, ADD, SUB)
                f2 = apool.tile([128, R], F32, tag="af")
                nc.vector.tensor_tensor(f2, x2, z2, SUB)
                a_t = apool.tile([128, R], F32R, tag="asin")
                nc.scalar.activation(a_t, f2, SIN, scale=TWO_PI)
                for r in range(2):
                    lhsT = a_t[:, r * 128 : (r + 1) * 128]
                    for h in range(2):
                        nc.tensor.matmul(
                            pmt[r][h],
                            lhsT,
                            xt_sb[:, kt, h * 512 : (h + 1) * 512],
                            start=(kt == 0),
                            stop=(kt == KT - 1),
                        )
            for r in range(2):
                for h in range(2):
                    # fold the +/-c_j * alpha/(d1*d2) column scale of A into
                    # the PSUM->SBUF copy (per-partition scalar), casting bf16
                    nc.vector.tensor_scalar(
                        tt_sb[:, r, h * 512 : (h + 1) * 512],
                        pmt[r][h],
                        mcols_sb[:, r : r + 1],
                        None,
                        MULT,
                    )

            # ---- B matrix: [2 x 128 rows, 4096] bf16, built in 512-col strips
            for t in range(2):
                for q in range(8):
                    xb = bpool.tile([128, 512], F32, tag="bx")
                    nc.vector.scalar_tensor_tensor(
                        xb,
                        lrow_sb[:],
                        vcols_sb[:, t : t + 1],
                        offc_sb[:, t * 8 + q : t * 8 + q + 1].to_broadcast(
                            (128, 512)
                        ),
                        MULT,
                        ADD,
                    )
                    zb = bpool.tile([128, 512], F32, tag="bz")
                    nc.vector.tensor_scalar(zb, xb, MAGIC, MAGIC, ADD, SUB)
                    fb = bpool.tile([128, 512], F32, tag="bf")
                    nc.vector.tensor_tensor(fb, xb, zb, SUB)
                    nc.scalar.activation(
                        b_sb[:, t, q * 512 : (q + 1) * 512], fb, SIN, scale=TWO_PI
                    )

            # ---- main loop: h^T[l, s] = sum_k wt[k, l]*xt[k, s] + delta + b
            for lo in range(16):  # 256 output rows (hT partitions) per iter
                pms = [[ppool.tile([128, 512], F32, tag="pm", name=f"pms_{lo}_{j}_{h}")
                        for h in range(2)] for j in range(2)]
                for kt in range(KT):
                    w_t = wpool.tile([128, 256], F32R, tag="w")
                    nc.sync.dma_start(
                        w_t,
                        wt_d[kt * 128 : (kt + 1) * 128, lo * 256 : (lo + 1) * 256],
                    )
                    for j in range(2):
                        lhsT = w_t[:, j * 128 : (j + 1) * 128]
                        for h in range(2):
                            nc.tensor.matmul(
                                pms[j][h],
                                lhsT,
                                xt_sb[:, kt, h * 512 : (h + 1) * 512],
                                start=(kt == 0),
                                stop=False,
                            )
                for r in range(2):
                    for j in range(2):
                        lhsT = b_sb[:, r, lo * 256 + j * 128 : lo * 256 + (j + 1) * 128]
                        for h in range(2):
                            nc.tensor.matmul(
                                pms[j][h],
                                lhsT,
                                tt_sb[:, r, h * 512 : (h + 1) * 512],
                                start=False,
                                stop=(r == 1),
                            )
                for j in range(2):
                    lsub = lo * 2 + j
                    ot = opool.tile([128, S_CORE], F32, tag="ot")
                    for h in range(2):
                        nc.scalar.activation(
                            ot[:, h * 512 : (h + 1) * 512],
                            pms[j][h],
                            IDENT,
                            bias=bias_sb[:, lsub : lsub + 1],
                            scale=1.0,
                        )
                    nc.sync.dma_start(
                        ht_d[lsub * 128 : (lsub + 1) * 128, :], ot
                    )

    nc.compile()
    return nc


def _host_prep(x, c, E, W_base, b_base):
    """Shard + lay out inputs. All index math is exact in f32 (< 2^24)."""
    x2d = np.ascontiguousarray(np.asarray(x, dtype=np.float32).reshape(S_TOTAL, D1))
    W = np.asarray(W_base, dtype=np.float32)
    b = np.asarray(b_base, dtype=np.float32)
    c32 = np.asarray(c, dtype=np.float32)
    u = np.asarray(E[0]).astype(np.float32)
    v = np.asarray(E[1]).astype(np.float32)

    s_fft = np.float32(ALPHA / (D1 * D2))

    u_r = np.zeros(R, np.float32)
    u_r[:NCOEF] = u
    u_r[NCOEF : 2 * NCOEF] = u
    delta_r = np.zeros(R, np.float32)
    delta_r[:NCOEF] = 0.25  # cos(x) = sin(x + pi/2): quarter-turn offset
    m_r = np.zeros(R, np.float32)
    m_r[:NCOEF] = c32 * s_fft
    m_r[NCOEF : 2 * NCOEF] = -c32 * s_fft
    v_r = np.zeros(R, np.float32)
    v_r[:NCOEF] = v
    v_r[NCOEF : 2 * NCOEF] = v
    cosrow_r = np.zeros(R, np.float32)
    cosrow_r[:NCOEF] = 0.25

    p = np.arange(128, dtype=np.float32)[:, None]
    u2 = np.tile(u_r[None, :], (128, 1))
    puq = (p * u_r[None, :]) * np.float32(INV4096) + delta_r[None, :]
    mcols = np.ascontiguousarray(m_r.reshape(2, 128).T)
    vcols = np.ascontiguousarray(v_r.reshape(2, 128).T)
    # offc[p, t*8+q] = q*v/8 + (0.25 if row t*128+p is a cos row)
    q_ix = np.arange(8, dtype=np.float32)
    offc = np.zeros((128, 16), np.float32)
    for t in range(2):
        vt = v_r[t * 128 : (t + 1) * 128][:, None]
        ct = cosrow_r[t * 128 : (t + 1) * 128][:, None]
        offc[:, t * 8 : (t + 1) * 8] = q_ix[None, :] * vt * np.float32(0.125) + ct
    lrow = np.tile(
        (np.arange(512, dtype=np.float32) * np.float32(INV4096))[None, :], (128, 1)
    )
    bias_cols = np.ascontiguousarray(b.reshape(32, 128).T)
    wt = np.ascontiguousarray(W.T)

    shared = {
        "wt": wt,
        "biasc": bias_cols,
        "u2": u2,
        "puq": puq,
        "mcols": mcols,
        "vcols": vcols,
        "offc": offc,
        "lrow": lrow,
    }
    in_maps = []
    for core in range(NCORES):
        xt = np.ascontiguousarray(x2d[core * S_CORE : (core + 1) * S_CORE, :].T)
        in_maps.append({"xt": xt, **shared})
    return in_maps


def get_nc():
    if "nc" not in _CACHE:
        _CACHE["nc"] = _build_nc()
    return _CACHE["nc"]


def run(inputs, trace=False):
    nc = get_nc()
    in_maps = _host_prep(
        inputs["x"], inputs["c"], inputs["E"], inputs["W_base"], inputs["b_base"]
    )
    res = bass_utils.run_bass_kernel_spmd(
        nc, in_maps, core_ids=list(range(NCORES)), trace=trace
    )
    h = np.empty((S_TOTAL, D2), np.float32)
    for core in range(NCORES):
        h[core * S_CORE : (core + 1) * S_CORE, :] = res.results[core]["ht"].T
    out = h.reshape(np.asarray(inputs["x"]).shape[:2] + (D2,))
    return out, res


def kernel(**inputs):
    out, _ = run(inputs)
    return out


# revision 6
# speedup vs baseline: 1.1025x; 1.1025x over previous
"""FourierFT fused kernel for Trainium2 (8 NeuronCores, SPMD data-parallel).

Computes h = x @ W_base^T + b_base + x @ Delta_W where
Delta_W = real(ifft2(scatter(c, E))) * ALPHA.

Key algebraic identity: with only N=100 nonzero spectral coefficients,
Delta_W[k, l] = s * sum_j c_j * cos(2*pi*(k*u_j + l*v_j)/4096)
             = (A @ B)[k, l]   with rank 2N:
  A[k, j]    =  c_j*s*cos(2*pi*k*u_j/4096)     B[j, l]    = cos(2*pi*l*v_j/4096)
  A[k, N+j]  = -c_j*s*sin(2*pi*k*u_j/4096)     B[N+j, l]  = sin(2*pi*l*v_j/4096)
so the dense iFFT is never materialized; the delta path is a rank-200 update
folded into the same PSUM accumulation as the base matmul.

Device layout: each core owns a 1024-row slice of x (flattened to [8192, 4096]),
pre-transposed on the host to k-major ([4096, 1024]) so the contraction dim sits
on SBUF partitions. W_base is host-transposed to [in, out] and replicated. The
output is produced as h^T tiles ([l, s]) so the bias is a per-partition scalar
and DMA rows stay 4 KB-contiguous; the host re-transposes the shards at the end.

Trig tables A and B are built on-device: the index products k*u and l*v stay
below 2^24 so all f32 index arithmetic is exact; range reduction to the Sin
LUT's [-pi, pi] window uses the round-to-nearest magic constant 2^23.
Matmuls run as float32r (FP22 multiplies, fp32 accumulate) which streams at
full PE rate for moving dims >= 256; B and t = x@A are kept in bf16 (their
contribution to h is ~0.6%, so bf16 there perturbs h by ~1e-5 relative).
"""

import sys

if "/opt/trn_rl_repo" not in sys.path:
    sys.path.insert(0, "/opt/trn_rl_repo")

import numpy as np

import concourse.bass as bass  # noqa: F401  (registers AP machinery)
import concourse.mybir as mybir
import concourse.tile as tile
from concourse import bacc, bass_utils

D1 = 4096
D2 = 4096
ALPHA = 300.0
NCOEF = 100
NCORES = 8
S_TOTAL = 4 * 2048
S_CORE = S_TOTAL // NCORES  # 1024
KT = D1 // 128  # 32 k-tiles
R = 256  # padded rank (cols 0..99 cos, 100..199 sin, rest zero)
MAGIC = float(2**23)
# one ulp below 2*pi: keeps f*scale inside the Sin LUT's [-pi, pi] window
# even at f = +/-0.5 exactly (f32(pi) rounds above float64 pi)
TWO_PI = float(np.nextafter(np.float32(2 * np.pi), np.float32(0)))
INV4096 = float(2**-12)

F32 = mybir.dt.float32
F32R = mybir.dt.float32r
BF16 = mybir.dt.bfloat16
MULT = mybir.AluOpType.mult
ADD = mybir.AluOpType.add
SUB = mybir.AluOpType.subtract
SIN = mybir.ActivationFunctionType.Sin
IDENT = mybir.ActivationFunctionType.Identity

_CACHE = {}


def _build_nc():
    """Trace + compile the single-core program (identical across cores)."""
    nc = bacc.Bacc("TRN2", target_bir_lowering=False, debug=False)

    xt_d = nc.dram_tensor("xt", [D1, S_CORE], F32R, kind="ExternalInput").ap()
    wt_d = nc.dram_tensor("wt", [D1, D2], F32R, kind="ExternalInput").ap()
    bias_d = nc.dram_tensor("biasc", [128, 32], F32, kind="ExternalInput").ap()
    u2_d = nc.dram_tensor("u2", [128, R], F32, kind="ExternalInput").ap()
    puq_d = nc.dram_tensor("puq", [128, R], F32, kind="ExternalInput").ap()
    mcols_d = nc.dram_tensor("mcols", [128, 2], F32, kind="ExternalInput").ap()
    vcols_d = nc.dram_tensor("vcols", [128, 2], F32, kind="ExternalInput").ap()
    offc_d = nc.dram_tensor("offc", [128, 16], F32, kind="ExternalInput").ap()
    lrow_d = nc.dram_tensor("lrow", [128, 512], F32, kind="ExternalInput").ap()
    ht_d = nc.dram_tensor("ht", [D2, S_CORE], F32, kind="ExternalOutput").ap()

    with tile.TileContext(nc) as tc:
        with (
            tc.tile_pool(name="resident", bufs=1) as rpool,
            tc.tile_pool(name="wstream", bufs=8) as wpool,
            tc.tile_pool(name="ascratch", bufs=2) as apool,
            tc.tile_pool(name="bscratch", bufs=2) as bpool,
            tc.tile_pool(name="outstage", bufs=3) as opool,
            tc.tile_pool(name="psum", bufs=8, space="PSUM") as ppool,
        ):
            # ---- resident tiles ----
            xt_sb = rpool.tile([128, KT, S_CORE], F32R, tag="xt")
            b_sb = rpool.tile([128, 2, D2], BF16, tag="bmat")
            tt_sb = rpool.tile([128, 2, S_CORE], BF16, tag="tt")
            lrow_sb = rpool.tile([128, 512], F32, tag="lrow")
            u2_sb = rpool.tile([128, R], F32, tag="u2")
            puq_sb = rpool.tile([128, R], F32, tag="puq")
            bias_sb = rpool.tile([128, 32], F32, tag="bias")
            mcols_sb = rpool.tile([128, 2], F32, tag="mcols")
            vcols_sb = rpool.tile([128, 2], F32, tag="vcols")
            offc_sb = rpool.tile([128, 16], F32, tag="offc")

            nc.sync.dma_start(u2_sb[:], u2_d[:])
            nc.sync.dma_start(puq_sb[:], puq_d[:])
            nc.sync.dma_start(mcols_sb[:], mcols_d[:])
            nc.sync.dma_start(vcols_sb[:], vcols_d[:])
            nc.sync.dma_start(offc_sb[:], offc_d[:])
            nc.sync.dma_start(lrow_sb[:], lrow_d[:])
            nc.sync.dma_start(bias_sb[:], bias_d[:])
            for kt in range(KT):
                nc.sync.dma_start(
                    xt_sb[:, kt, :], xt_d[kt * 128 : (kt + 1) * 128, :]
                )

            # ---- prefix: A tiles -> t^T MMs, interleaved with main lo=0 so
            # the PE stays fed while xt streams in (8 MMs per arriving k-tile)
            pmt = [[ppool.tile([128, 512], F32, tag="pm", name=f"pmt_{r}_{h}")
                    for h in range(2)] for r in range(2)]
            pms0 = [[ppool.tile([128, 512], F32, tag="pm", name=f"pms0_{j}_{h}")
                     for h in range(2)] for j in range(2)]
            for kt in range(KT):
                x2 = apool.tile([128, R], F32, tag="ax")
                nc.vector.scalar_tensor_tensor(
                    x2, u2_sb[:], float(kt * 128) * INV4096, puq_sb[:], MULT, ADD
                )
                z2 = apool.tile([128, R], F32, tag="az")
                nc.vector.tensor_scalar(z2, x2, MAGIC, MAGIC, ADD, SUB)
                f2 = apool.tile([128, R], F32, tag="af")
                nc.vector.tensor_tensor(f2, x2, z2, SUB)
                a_t = apool.tile([128, R], F32R, tag="asin")
                nc.scalar.activation(a_t, f2, SIN, scale=TWO_PI)
                for r in range(2):
                    lhsT = a_t[:, r * 128 : (r + 1) * 128]
                    for h in range(2):
                        nc.tensor.matmul(
                            pmt[r][h],
                            lhsT,
                            xt_sb[:, kt, h * 512 : (h + 1) * 512],
                            start=(kt == 0),
                            stop=(kt == KT - 1),
                        )
                w_t = wpool.tile([128, 256], F32R, tag="w", name=f"w0_{kt}")
                nc.sync.dma_start(
                    w_t, wt_d[kt * 128 : (kt + 1) * 128, 0:256]
                )
                for j in range(2):
                    lhsT = w_t[:, j * 128 : (j + 1) * 128]
                    for h in range(2):
                        nc.tensor.matmul(
                            pms0[j][h],
                            lhsT,
                            xt_sb[:, kt, h * 512 : (h + 1) * 512],
                            start=(kt == 0),
                            stop=False,
                        )
            for r in range(2):
                for h in range(2):
                    # fold the +/-c_j * alpha/(d1*d2) column scale of A into
                    # the PSUM->SBUF copy (per-partition scalar), casting bf16
                    nc.vector.tensor_scalar(
                        tt_sb[:, r, h * 512 : (h + 1) * 512],
                        pmt[r][h],
                        mcols_sb[:, r : r + 1],
                        None,
                        MULT,
                    )

            # ---- B matrix: [2 x 128 rows, 4096] bf16, built in 512-col strips
            for t in range(2):
                for q in range(8):
                    xb = bpool.tile([128, 512], F32, tag="bx")
                    nc.vector.scalar_tensor_tensor(
                        xb,
                        lrow_sb[:],
                        vcols_sb[:, t : t + 1],
                        offc_sb[:, t * 8 + q : t * 8 + q + 1].to_broadcast(
                            (128, 512)
                        ),
                        MULT,
                        ADD,
                    )
                    zb = bpool.tile([128, 512], F32, tag="bz")
                    nc.vector.tensor_scalar(zb, xb, MAGIC, MAGIC, ADD, SUB)
                    fb = bpool.tile([128, 512], F32, tag="bf")
                    nc.vector.tensor_tensor(fb, xb, zb, SUB)
                    nc.scalar.activation(
                        b_sb[:, t, q * 512 : (q + 1) * 512], fb, SIN, scale=TWO_PI
                    )

            # ---- main loop: h^T[l, s] = sum_k wt[k, l]*xt[k, s] + delta + b
            for lo in range(16):  # 256 output rows (hT partitions) per iter
                if lo == 0:
                    pms = pms0
                else:
                    pms = [[ppool.tile([128, 512], F32, tag="pm",
                                       name=f"pms_{lo}_{j}_{h}")
                            for h in range(2)] for j in range(2)]
                    for kt in range(KT):
                        w_t = wpool.tile([128, 256], F32R, tag="w")
                        nc.sync.dma_start(
                            w_t,
                            wt_d[kt * 128 : (kt + 1) * 128,
                                 lo * 256 : (lo + 1) * 256],
                        )
                        for j in range(2):
                            lhsT = w_t[:, j * 128 : (j + 1) * 128]
                            for h in range(2):
                                nc.tensor.matmul(
                                    pms[j][h],
                                    lhsT,
                                    xt_sb[:, kt, h * 512 : (h + 1) * 512],
                                    start=(kt == 0),
                                    stop=False,
                                )
                for r in range(2):
                    for j in range(2):
                        lhsT = b_sb[:, r, lo * 256 + j * 128 : lo * 256 + (j + 1) * 128]
                        for h in range(2):
                            nc.tensor.matmul(
                                pms[j][h],
                                lhsT,
                                tt_sb[:, r, h * 512 : (h + 1) * 512],
                                start=False,
                                stop=(r == 1),
                            )
                for j in range(2):
                    lsub = lo * 2 + j
                    ot = opool.tile([128, S_CORE], F32, tag="ot")
                    for h in range(2):
                        nc.scalar.activation(
                            ot[:, h * 512 : (h + 1) * 512],
                            pms[j][h],
                            IDENT,
                            bias=bias_sb[:, lsub : lsub + 1],
                            scale=1.0,
                        )
                    nc.sync.dma_start(
                        ht_d[lsub * 128 : (lsub + 1) * 128, :], ot
                    )

    nc.compile()
    return nc


def _host_prep(x, c, E, W_base, b_base):
    """Shard + lay out inputs. All index math is exact in f32 (< 2^24)."""
    x2d = np.ascontiguousarray(np.asarray(x, dtype=np.float32).reshape(S_TOTAL, D1))
    W = np.asarray(W_base, dtype=np.float32)
    b = np.asarray(b_base, dtype=np.float32)
    c32 = np.asarray(c, dtype=np.float32)
    u = np.asarray(E[0]).astype(np.float32)
    v = np.asarray(E[1]).astype(np.float32)

    s_fft = np.float32(ALPHA / (D1 * D2))

    u_r = np.zeros(R, np.float32)
    u_r[:NCOEF] = u
    u_r[NCOEF : 2 * NCOEF] = u
    delta_r = np.zeros(R, np.float32)
    delta_r[:NCOEF] = 0.25  # cos(x) = sin(x + pi/2): quarter-turn offset
    m_r = np.zeros(R, np.float32)
    m_r[:NCOEF] = c32 * s_fft
    m_r[NCOEF : 2 * NCOEF] = -c32 * s_fft
    v_r = np.zeros(R, np.float32)
    v_r[:NCOEF] = v
    v_r[NCOEF : 2 * NCOEF] = v
    cosrow_r = np.zeros(R, np.float32)
    cosrow_r[:NCOEF] = 0.25

    p = np.arange(128, dtype=np.float32)[:, None]
    u2 = np.tile(u_r[None, :], (128, 1))
    puq = (p * u_r[None, :]) * np.float32(INV4096) + delta_r[None, :]
    mcols = np.ascontiguousarray(m_r.reshape(2, 128).T)
    vcols = np.ascontiguousarray(v_r.reshape(2, 128).T)
    # offc[p, t*8+q] = q*v/8 + (0.25 if row t*128+p is a cos row)
    q_ix = np.arange(8, dtype=np.float32)
    offc = np.zeros((128, 16), np.float32)
    for t in range(2):
        vt = v_r[t * 128 : (t + 1) * 128][:, None]
        ct = cosrow_r[t * 128 : (t + 1) * 128][:, None]
        offc[:, t * 8 : (t + 1) * 8] = q_ix[None, :] * vt * np.float32(0.125) + ct
    lrow = np.tile(
        (np.arange(512, dtype=np.float32) * np.float32(INV4096))[None, :], (128, 1)
    )
    bias_cols = np.ascontiguousarray(b.reshape(32, 128).T)
    wt = np.ascontiguousarray(W.T)

    shared = {
        "wt": wt,
        "biasc": bias_cols,
        "u2": u2,
        "puq": puq,
        "mcols": mcols,
        "vcols": vcols,
        "offc": offc,
        "lrow": lrow,
    }
    in_maps = []
    for core in range(NCORES):
        xt = np.ascontiguousarray(x2d[core * S_CORE : (core + 1) * S_CORE, :].T)
        in_maps.append({"xt": xt, **shared})
    return in_maps


def get_nc():
    if "nc" not in _CACHE:
        _CACHE["nc"] = _build_nc()
    return _CACHE["nc"]


def run(inputs, trace=False):
    nc = get_nc()
    in_maps = _host_prep(
        inputs["x"], inputs["c"], inputs["E"], inputs["W_base"], inputs["b_base"]
    )
    res = bass_utils.run_bass_kernel_spmd(
        nc, in_maps, core_ids=list(range(NCORES)), trace=trace
    )
    h = np.empty((S_TOTAL, D2), np.float32)
    for core in range(NCORES):
        h[core * S_CORE : (core + 1) * S_CORE, :] = res.results[core]["ht"].T
    out = h.reshape(np.asarray(inputs["x"]).shape[:2] + (D2,))
    return out, res


def kernel(**inputs):
    out, _ = run(inputs)
    return out
